# revision 1
# baseline (speedup 1.0000x reference)
"""Causal self-attention (non-masked softmax path) for TRN2, 8 NeuronCores.

Sharding: 2-way data parallel over batch x 4-way tensor parallel over heads.
Core c handles batch b = c // 4, head group g = c % 4 (heads 4g..4g+3).
Each core computes its QKV projection slice, full attention for its 4 heads,
and the row-parallel c_proj partial; the host sums the 4 partials per batch
(the all-reduce of row-parallel tensor parallelism) and adds b_proj.

All matmuls take bf16 inputs (fp32 matmul on TRN2 runs as 2 passes at 2
bytes/cycle = 4x slower) and accumulate in fp32 PSUM. Measured end-to-end
error vs the fp32 reference is ~6e-3 scale-relative.

Per-core dataflow:
  xT [1024, 2048] bf16 (host pre-transposed; matmuls contract over partitions)
  qkT = Wqk.T @ x.T [512, 2048] bf16, q pre-scaled by 1/sqrt(hd) via W on host
  v   = x @ Wv [2048, 4 x (64|1)] bf16 -- augmented with a ones column per
        head so the PV matmul (lhsT [128, 65]) also produces the softmax
        denominator in psum row 64.
  S^T chunks [128 k, 512 q] per head, K=64 row-packed 2 heads per PE pass
  (row-group packing runs concurrently; col-group packing does not).
  exp on ACT over pair-fused [128, 1024] psum -> bf16 E tiles.
  Normalize y^T after PV: stage pvd to SBUF (frees psum banks fast), batched
  reciprocal of the 4 denominator rows, selector-matmul broadcast, DVE mul.
  c_proj contracts the 256 group features; partial out stays fp32.

A single PSUM pool with tags "s" (2 x 2 banks) and "pvd" (4 x 1 bank) is
shared by all phases so projection, attention, and c_proj pipeline into
each other instead of serializing on pool-scope boundaries. PSUM->SBUF
copies that would collide with DVE work run on the otherwise-idle ACT
engine (activation Copy/Identity).
"""

import numpy as np

B, T, H, NH, HD = 2, 2048, 1024, 16, 64
P = 128
FG = 256          # features per head group (4 heads x 64)
VC = 65           # v columns per head incl. the ones column
NQ = 512          # Tq chunk (psum free dim)
NJ = T // NQ      # 4
NI = T // P       # 16 key chunks
KH = H // P       # 8 hidden chunks
NCORES = 8

_CACHE = {}


def _build():
    import concourse.bacc as bacc
    import concourse.mybir as mybir
    import concourse.tile as tile

    fp32 = mybir.dt.float32
    bf16 = mybir.dt.bfloat16

    nc = bacc.Bacc("TRN2", debug=False)
    xT = nc.dram_tensor("xT", [H, T], bf16, kind="ExternalInput").ap()
    wqkv = nc.dram_tensor("wqkv", [H, 3 * FG], bf16, kind="ExternalInput").ap()
    bqk = nc.dram_tensor("bqk", [2 * FG], fp32, kind="ExternalInput").ap()
    bv = nc.dram_tensor("bv", [FG], bf16, kind="ExternalInput").ap()
    wp = nc.dram_tensor("wp", [FG, H], bf16, kind="ExternalInput").ap()
    out = nc.dram_tensor("out", [T, H], fp32, kind="ExternalOutput").ap()

    with tile.TileContext(nc) as tc:
        _emit(nc, tc, mybir, xT, wqkv, bqk, bv, wp, out)
    nc.compile()
    return nc


def _emit(nc, tc, mybir, xT, wqkv, bqk, bv, wp, out):
    from contextlib import ExitStack

    fp32 = mybir.dt.float32
    bf16 = mybir.dt.bfloat16
    Exp = mybir.ActivationFunctionType.Exp
    Ident = mybir.ActivationFunctionType.Identity
    Copy = mybir.ActivationFunctionType.Copy

    W3 = 3 * FG  # 768, wqkv row width

    with ExitStack() as ctx:
        pool = lambda name, bufs=1, space="SBUF": ctx.enter_context(
            tc.tile_pool(name=name, bufs=bufs, space=space)
        )

        const = pool("const")
        ones = const.tile([1, P], bf16)
        nc.vector.memset(ones[:], 1.0)
        bias0 = const.tile([P, 1], fp32)
        nc.vector.memset(bias0[:], 0.0)
        # selector weights for the denominator broadcast: sel[k, 64h+m]
        # = 1 iff k == 32h; sel_h.T @ rcp replicates head h's denominator
        # row (32h of rcp) across 64 output partitions starting at base 0.
        sel = const.tile([P, 4 * 64], bf16)
        nc.vector.memset(sel[:], 0.0)
        for h in range(4):
            nc.vector.memset(sel[32 * h : 32 * h + 1, h * 64 : (h + 1) * 64], 1.0)
        bqk_sb = const.tile([P, 4], fp32)
        nc.sync.dma_start(bqk_sb[:], bqk.rearrange("(m p) -> p m", p=P))
        bv_sb = const.tile([1, FG], bf16)
        nc.sync.dma_start(bv_sb[:], bv.rearrange("(o f) -> o f", o=1))

        xt_sb = pool("xt").tile([P, KH * T], bf16)
        w_sb = pool("w").tile([P, KH * W3], bf16)
        wp_sb = pool("wp").tile([P, 2 * H], bf16)
        for k in range(KH):
            nc.sync.dma_start(xt_sb[:, k * T : (k + 1) * T], xT[k * P : (k + 1) * P, :])
            nc.sync.dma_start(
                w_sb[:, k * W3 : (k + 1) * W3], wqkv[k * P : (k + 1) * P, :]
            )
        for kk in range(2):
            nc.sync.dma_start(
                wp_sb[:, kk * H : (kk + 1) * H], wp[kk * P : (kk + 1) * P, :]
            )

        qk_sb = pool("qk").tile([P, 4 * T], bf16)   # feat chunks: q0 q1 k0 k1
        # v weight stride padded to 128 columns so PV ldweights gets FWL
        # (4x faster weight load); cols 65..127 are zeros -> junk psum rows
        v_sb = pool("v").tile([P, NI * 4 * P], bf16)
        v4 = v_sb.rearrange("p (t h c) -> p t h c", t=NI, h=4, c=P)
        nc.vector.memset(v_sb[:], 0.0)
        nc.vector.memset(v4[:, :, :, 64:65], 1.0)   # denominator ones columns
        y_sb = pool("y").tile([P, 2 * T], bf16)     # y^T, feat pair chunks x T

        # one psum pool for all phases: tag "s" = 2 slots x 2 banks,
        # tag "pvd" = 4 slots x 1 bank (16KB/partition total)
        ps_pool = ctx.enter_context(tc.tile_pool(name="ps", bufs=2, space="PSUM"))
        epool = pool("e", bufs=3)
        nrm = pool("nrm", bufs=2)
        pvs_pool = pool("pvs", bufs=4)
        outp = pool("outp", bufs=3)

        # ---- Phase A: projections (pair-0 k/q first so attention starts early)
        for m in (2, 0, 3, 1):
            for j in range(NJ):
                ps = ps_pool.tile([P, 2 * NQ], fp32, tag="s", name=f"qk{m}_{j}")
                for k in range(KH):
                    nc.tensor.matmul(
                        ps[:, 0:NQ],
                        w_sb[:, k * W3 + m * P : k * W3 + (m + 1) * P],
                        xt_sb[:, k * T + j * NQ : k * T + (j + 1) * NQ],
                        start=(k == 0),
                        stop=(k == KH - 1),
                    )
                nc.scalar.activation(  # copy+bias on ACT (idle during phase A)
                    qk_sb[:, m * T + j * NQ : m * T + (j + 1) * NQ],
                    ps[:, 0:NQ],
                    Ident,
                    bias=bqk_sb[:, m : m + 1],
                )
        for t in range(NI):  # v in natural layout, strided into [64|1] slots
            ps = ps_pool.tile([P, NQ], fp32, tag="pvd", bufs=4, name=f"v{t}")
            for k in range(KH):
                nc.tensor.matmul(
                    ps[:, 0:FG],
                    xt_sb[:, k * T + t * P : k * T + (t + 1) * P],
                    w_sb[:, k * W3 + 2 * FG : (k + 1) * W3],
                    start=(k == 0),
                    stop=False,
                )
            nc.tensor.matmul(  # += ones.T @ bv  (bias broadcast over rows)
                ps[:, 0:FG],
                ones[0:1, :],
                bv_sb[0:1, :],
                start=False,
                stop=True,
            )
            nc.vector.tensor_copy(
                v4[:, t, :, 0:64],
                ps[:, 0:FG].rearrange("p (h c) -> p h c", h=4, c=64),
            )

        # ---- Phase B: attention ----
        for j in range(NJ):
            pvd = [
                ps_pool.tile([P, NQ], fp32, tag="pvd", bufs=4, name=f"pvd{j}_{h}")
                for h in range(4)
            ]
            for i in range(NI):
                for p in range(2):  # head pairs (2p, 2p+1)
                    sp = ps_pool.tile([P, 2 * NQ], fp32, tag="s", name=f"s{j}_{i}_{p}")
                    for hh in range(2):
                        bp = 64 * hh
                        # S^T chunk: K=64, row-packed -> runs concurrently
                        nc.tensor.matmul(
                            sp[:, hh * NQ : (hh + 1) * NQ],
                            qk_sb[
                                bp : bp + 64,
                                (2 + p) * T + i * P : (2 + p) * T + (i + 1) * P,
                            ],
                            qk_sb[
                                bp : bp + 64, p * T + j * NQ : p * T + (j + 1) * NQ
                            ],
                            start=True,
                            stop=True,
                            tile_position=(bp, 0),
                        )
                    e = epool.tile([P, 2 * NQ], bf16, tag="e")
                    nc.scalar.activation(e[:], sp[:], Exp, bias=bias0[:, 0:1])
                    for hh in range(2):
                        h = 2 * p + hh
                        nc.tensor.matmul(  # y^T rows 0..63, denominator row 64
                            pvd[h][:],
                            v4[:, i, h, :],
                            e[:, hh * NQ : (hh + 1) * NQ],
                            start=(i == 0),
                            stop=(i == NI - 1),
                        )
            # normalize: stage to SBUF (frees pvd banks), batched reciprocal,
            # selector broadcast, all-SBUF multiplies
            pvs = [
                pvs_pool.tile([VC, NQ], fp32, tag="pvs", name=f"pvs{j}_{h}")
                for h in range(4)
            ]
            for h in range(4):
                nc.vector.tensor_copy(pvs[h][:], pvd[h][0:VC, :])
            dg = nrm.tile([P, NQ], fp32, tag="dg")
            nc.vector.memset(dg[:], 1.0)  # keep untouched rows finite
            for h in range(4):
                nc.vector.tensor_copy(dg[32 * h : 32 * h + 1, :], pvs[h][64:65, :])
            rcp = nrm.tile([P, NQ], bf16, tag="rcp")
            with nc.allow_low_precision(reason="softmax denom broadcast in bf16"):
                nc.vector.reciprocal(rcp[:], dg[:])
            for h in range(4):
                p, hh = divmod(h, 2)
                bc = ps_pool.tile([64, NQ], fp32, tag="pvd", bufs=4, name=f"bc{j}_{h}")
                nc.tensor.matmul(
                    bc[:], sel[:, h * 64 : (h + 1) * 64], rcp[:], start=True, stop=True
                )
                bc_sb = nrm.tile([64, NQ], fp32, tag="bcs")
                nc.scalar.activation(bc_sb[:], bc[:], Copy)
                nc.vector.tensor_mul(  # both SB inputs at base partition 0
                    y_sb[
                        64 * hh : 64 * hh + 64,
                        p * T + j * NQ : p * T + (j + 1) * NQ,
                    ],
                    pvs[h][0:64, :],
                    bc_sb[:],
                )

        # ---- Phase C: c_proj partial (row-parallel) ----
        for mq in range(NI):
            pcs = [
                ps_pool.tile([P, NQ], fp32, tag="pvd", bufs=4, name=f"c{mq}_{n}")
                for n in range(2)
            ]
            for kk in range(2):  # kk outer: each y lhsT loads once for both n
                for n in range(2):
                    nc.tensor.matmul(
                        pcs[n][:],
                        y_sb[:, kk * T + mq * P : kk * T + (mq + 1) * P],
                        wp_sb[:, kk * H + n * NQ : kk * H + (n + 1) * NQ],
                        start=(kk == 0),
                        stop=(kk == 1),
                    )
            for n in range(2):
                ot = outp.tile([P, NQ], fp32, tag="o")
                nc.scalar.activation(ot[:], pcs[n][:], Copy)  # ACT idle in phase C
                nc.sync.dma_start(
                    out[mq * P : (mq + 1) * P, n * NQ : (n + 1) * NQ], ot[:]
                )


def _get_nc():
    if "nc" not in _CACHE:
        _CACHE["nc"] = _build()
    return _CACHE["nc"]


def _make_in_maps(x, W_attn, b_attn, W_proj):
    import ml_dtypes

    bf = ml_dtypes.bfloat16
    x = np.asarray(x, np.float32)
    W_attn = np.asarray(W_attn, np.float32)
    b_attn = np.asarray(b_attn, np.float32)
    W_proj = np.asarray(W_proj, np.float32)
    scale = 1.0 / np.sqrt(np.float32(HD))
    in_maps = []
    for c in range(NCORES):
        b, g = divmod(c, 4)
        sl = slice(FG * g, FG * (g + 1))
        wq = W_attn[:, sl] * scale
        wk = W_attn[:, H:][:, sl]
        wv = W_attn[:, 2 * H :][:, sl]
        in_maps.append(
            {
                "xT": np.ascontiguousarray(x[b].T).astype(bf),
                "wqkv": np.ascontiguousarray(
                    np.concatenate([wq, wk, wv], axis=1)
                ).astype(bf),
                "bqk": np.concatenate(
                    [b_attn[sl] * scale, b_attn[H:][sl]]
                ).astype(np.float32),
                "bv": np.ascontiguousarray(b_attn[2 * H :][sl]).astype(bf),
                "wp": np.ascontiguousarray(W_proj[sl, :]).astype(bf),
            }
        )
    return in_maps


def _gather(results, b_proj):
    b_proj = np.asarray(b_proj, np.float32)
    y = np.empty((B, T, H), np.float32)
    for b in range(B):
        acc = results[4 * b]["out"].astype(np.float32)
        for g in range(1, 4):
            acc = acc + results[4 * b + g]["out"]
        y[b] = acc + b_proj[None, :]
    return y


def run(x, W_attn, b_attn, W_proj, b_proj, trace=False):
    from concourse.bass_utils import run_bass_kernel_spmd

    nc = _get_nc()
    in_maps = _make_in_maps(x, W_attn, b_attn, W_proj)
    res = run_bass_kernel_spmd(nc, in_maps, list(range(NCORES)), trace=trace)
    return _gather(res.results, b_proj), res


def kernel(x, W_attn, b_attn, W_proj, b_proj):
    y, _ = run(x, W_attn, b_attn, W_proj, b_proj, trace=False)
    return y



# revision 10
# speedup vs baseline: 1.0568x; 1.0568x over previous
"""Causal self-attention (non-masked softmax) for TRN2, 8 NeuronCores.

Sharding: 2-way data parallel over batch x 4-way tensor parallel over heads.
Core c: batch c//4, head group c%4 (4 heads, 256 features). Host sums the
4 row-parallel c_proj partials per batch (fp16 partials) and adds the
host-folded biases (b_proj + b_attn_v @ W_proj; v-bias commutes through
softmax since sum_k p_k = 1).

Engine split per core:
  PE:  QKV projections, S^T (row-packed head pairs), PV (v augmented with a
       ones column so psum row 64 is the softmax denominator), c_proj.
  ACT: exp over S^T psum tiles [128,1024] (the kernel bottleneck) + the
       qk bias copies early (ACT idles during the projection lead-in).
  DVE: a slice of the exp tiles via two custom ops (compound base
       (1+s/256+s^2/(2*256^2)) then 8 chained squarings; ~6e-3 rel err at
       bf16 out), v staging copies, denominator reciprocal
       (reciprocal_approx_fast on the psum row), y normalize muls,
       c_proj psum->fp16 staging.
  GPSIMD: partition_broadcast of 1/denom rows, out-DMA issue.

PSUM: tag s = 2x[128,1024] (S tiles, double buffered), pvd = 2x[128,512]
(PV accumulators for the in-flight head pair), fill = 2x[128,512]
(projection / c_proj scratch) = 8 banks total.

Attention runs as 8 groups (j, pair), 16 key-chunk steps each; projection
and c_proj work is interleaved into the group streams as fillers to keep
the PE p-state high and ACT fed.
"""

import numpy as np

B, T, H, NH, HD = 2, 2048, 1024, 16, 64
P, FG = 128, 256
NQ = 512          # query block per j
NJ = 4            # T/NQ
NI = 16           # key chunks of 128
KH = 8            # hidden chunks of 128
W3 = 3 * FG       # wqkv row width per core (768)
NCORES = 8

# i-steps per (j,pair) group whose exp runs on DVE instead of ACT
DVE_EXP_I = (5, 10, 15)

_CACHE = {}


def _register_dve_exp():
    """Register the custom DVE exp ops (idempotent). exp(s) ~= base^(256)
    with base = 1 + s/256 + (s/256)^2/2: op1 computes the base, op2 squares
    eight times. Validated on hw: max rel err ~6e-3 over s in [-8, 8]."""
    import concourse.dve_ops as dve_ops
    from concourse.dve_ops import DveOp, OPS, CUSTOM_DVE_SPECS, _SUB_OPCODE_FOR_NAME
    from concourse.dve_spec import Spec, Src0, C0, C1, C2, lower, sq
    from concourse.dve_uop import DveOpSpec

    have = {o.name: o for o in OPS}
    if "EXP_BASE_XK" in have:
        return have["EXP_BASE_XK"], have["EXP_SQ8_XK"]

    def make(name, body, ref):
        spec = Spec(body=body, reference=ref)
        row = max(_SUB_OPCODE_FOR_NAME.values()) + 1
        _SUB_OPCODE_FOR_NAME[name] = row
        shas = {}
        for ver in ("v3",):
            tmp = DveOpSpec(name=name, opcode=row, uops=lower(spec, ver=ver),
                            rd1_en=False)
            shas[ver] = tmp.sha(ver)
        op = DveOp(name, spec, subdim=False, uops_sha=shas)
        OPS.append(op)
        CUSTOM_DVE_SPECS[name] = spec
        return op

    w = Src0 * C0
    base_body = C1 + w * (C1 + w * C2)

    def ref_base(in0, in1, s0, s1, imm2):
        ww = in0.astype(np.float32) * np.float32(s0)
        return (np.float32(s1) + ww * (np.float32(s1) + ww * np.float32(imm2))
                ).astype(np.float32)

    x = Src0
    for _ in range(8):
        x = sq(x)

    def ref_sq8(in0, in1, s0, s1, imm2):
        v = in0.astype(np.float32)
        for _ in range(8):
            v = (v * v).astype(np.float32)
        return v

    return make("EXP_BASE_XK", base_body, ref_base), make("EXP_SQ8_XK", x, ref_sq8)


def _build():
    import concourse.bacc as bacc
    import concourse.mybir as mybir
    import concourse.tile as tile

    EXP_BASE, EXP_SQ8 = _register_dve_exp()

    fp32 = mybir.dt.float32
    fp16 = mybir.dt.float16
    bf16 = mybir.dt.bfloat16

    nc = bacc.Bacc("TRN2", debug=False)
    xT = nc.dram_tensor("xT", [H, T], bf16, kind="ExternalInput").ap()
    wqkv = nc.dram_tensor("wqkv", [H, W3], bf16, kind="ExternalInput").ap()
    bqk = nc.dram_tensor("bqk", [2 * FG], fp32, kind="ExternalInput").ap()
    wp = nc.dram_tensor("wp", [FG, H], bf16, kind="ExternalInput").ap()
    out = nc.dram_tensor("out", [T, H], fp16, kind="ExternalOutput").ap()

    with tile.TileContext(nc) as tc:
        _emit(nc, tc, mybir, EXP_BASE, EXP_SQ8, xT, wqkv, bqk, wp, out)
    nc.compile()
    return nc


def _emit(nc, tc, mybir, EXP_BASE, EXP_SQ8, xT, wqkv, bqk, wp, out):
    from contextlib import ExitStack

    fp32 = mybir.dt.float32
    fp16 = mybir.dt.float16
    bf16 = mybir.dt.bfloat16
    Exp = mybir.ActivationFunctionType.Exp
    Ident = mybir.ActivationFunctionType.Identity

    with ExitStack() as ctx:
        pool = lambda name, bufs=1, space="SBUF": ctx.enter_context(
            tc.tile_pool(name=name, bufs=bufs, space=space)
        )

        # ---- persistent SBUF tiles (fine-grained so deps stay fine) ----
        const = pool("const")
        bias0 = const.tile([P, 1], fp32)
        nc.vector.memset(bias0[:], 0.0)
        junk = const.tile([P, NQ], bf16)        # PE warmup operand
        nc.vector.memset(junk[:], 0.001)
        warm = const.tile([P, 8], fp32)
        nc.vector.memset(warm[:], 0.0)
        bqk_sb = const.tile([P, 4], fp32)       # bias col per m-chunk
        nc.sync.dma_start(bqk_sb[:], bqk.rearrange("(m p) -> p m", p=P))

        xtp = pool("xt")
        # per (hidden chunk, token half) so early readers don't wait all DMA
        xts = [[xtp.tile([P, T // 2], bf16, name=f"xt{k}_{h}") for h in range(2)] for k in range(KH)]
        wqp = pool("wq")
        wqs = [wqp.tile([P, W3], bf16, name=f"wq{k}") for k in range(KH)]
        wpp = pool("wpp")
        wps = [wpp.tile([P, H], bf16, name=f"wp{k}") for k in range(2)]
        qkp = pool("qk")
        qks = [[qkp.tile([P, NQ], bf16, name=f"qk{m}_{j}") for j in range(NJ)] for m in range(4)]
        vp = pool("v")
        vts = [vp.tile([P, 4 * P], bf16, name=f"v{t}") for t in range(NI)]
        yp = pool("y")
        ys = [[yp.tile([P, NQ], bf16, name=f"y{k}_{j}") for j in range(NJ)] for k in range(2)]

        # ---- working pools ----
        epool = pool("e", bufs=4)
        scrp = pool("scr", bufs=2)      # DVE exp fp32 intermediate
        pvsp = pool("pvs", bufs=4)      # staged PV numerators
        rcpp = pool("rcp", bufs=4)      # 1/denom rows + broadcasts
        outp = pool("o", bufs=4)        # fp16 out staging
        ps = ctx.enter_context(tc.tile_pool(name="ps", bufs=2, space="PSUM"))

        # ---- warmups (off critical path): ACT exp table, PE p-state ----
        nc.scalar.activation(warm[:], warm[:], Exp, bias=bias0[:, 0:1])
        for t in range(NI):  # mark ones col + zero pad of v tiles
            v4 = vts[t].rearrange("p (h c) -> p h c", h=4, c=P)
            nc.vector.memset(v4[:, :, 64:66], 0.0)
            nc.vector.memset(v4[:, :, 64:65], 1.0)

        # ---- input DMAs, spread across idle engine queues ----
        dq = [nc.sync, nc.scalar, nc.gpsimd]
        di = [0]

        def dma(dst, src):
            dq[di[0] % 3].dma_start(dst, src)
            di[0] += 1

        for k in range(KH):  # critical prefix: w chunk + first token half
            dma(wqs[k][:], wqkv[k * P : (k + 1) * P, :])
            dma(xts[k][0][:], xT[k * P : (k + 1) * P, 0 : 2 * NQ])
        for k in range(KH):
            dma(xts[k][1][:], xT[k * P : (k + 1) * P, 2 * NQ :])
        for kk in range(2):
            dma(wps[kk][:], wp[kk * P : (kk + 1) * P, :])

        # PE warmup: ~14 junk matmuls to ramp the p-state while DMA lands
        for w in range(14):
            pw = ps.tile([P, NQ], fp32, tag="fill", name=f"warm{w}")
            nc.tensor.matmul(pw[:], junk[:, 0:P], junk[:], start=True, stop=True)

        # ---- building blocks ----
        def proj_qk(m, jpair):
            """q/k projection for feature chunk m, j-blocks (2*jpair, +1).
            Weights per (m,k) load once and serve both j matmuls; bias-add
            copies run on ACT (idle early) -> qk tiles."""
            pj = [
                ps.tile([P, NQ], fp32, tag="fill", name=f"qk{m}_{jpair}_{jj}")
                for jj in range(2)
            ]
            for k in range(KH):
                for jj in range(2):
                    nc.tensor.matmul(
                        pj[jj][:],
                        wqs[k][:, m * P : (m + 1) * P],
                        xts[k][jpair][:, jj * NQ : (jj + 1) * NQ],
                        start=(k == 0),
                        stop=(k == KH - 1),
                    )
            for jj in range(2):
                j = 2 * jpair + jj
                nc.scalar.activation(
                    qks[m][j][:], pj[jj][:], Ident, bias=bqk_sb[:, m : m + 1]
                )

        def proj_v(t):
            """v for token chunk t -> vts[t] cols [64h,64h+64) per head."""
            pv = ps.tile([P, NQ], fp32, tag="fill", name=f"v{t}")
            th, tl = divmod(t, 8)
            for k in range(KH):
                nc.tensor.matmul(
                    pv[:, 0:FG],
                    xts[k][th][:, tl * P : (tl + 1) * P],
                    wqs[k][:, 2 * FG :],
                    start=(k == 0),
                    stop=(k == KH - 1),
                )
            v4 = vts[t].rearrange("p (h c) -> p h c", h=4, c=P)
            nc.vector.tensor_copy(
                v4[:, :, 0:64], pv[:, 0:FG].rearrange("p (h c) -> p h c", h=4, c=64)
            )

        def cproj(j, mql):
            """c_proj for token rows j*512+128*mql."""
            pcs = [
                ps.tile([P, NQ], fp32, tag="fill", name=f"c{j}_{mql}_{n}")
                for n in range(2)
            ]
            for kk in range(2):
                for n in range(2):
                    nc.tensor.matmul(
                        pcs[n][:],
                        ys[kk][j][:, mql * P : (mql + 1) * P],
                        wps[kk][:, n * NQ : (n + 1) * NQ],
                        start=(kk == 0),
                        stop=(kk == 1),
                    )
            r0 = j * NQ + mql * P
            for n in range(2):
                ot = outp.tile([P, NQ], fp16, tag="o", name=f"ot{j}_{mql}_{n}")
                nc.vector.tensor_copy(ot[:], pcs[n][:])
                nc.sync.dma_start(out[r0 : r0 + P, n * NQ : (n + 1) * NQ], ot[:])

        # ---- filler schedule: callables interleaved into group streams ----
        def F(fn, *a):
            return lambda: fn(*a)

        fillers = {
            (0, 0): [F(proj_v, t) for t in range(2, 16)]
            + [F(proj_qk, 3, 0), F(proj_qk, 1, 0)],
            (0, 1): [F(proj_qk, 3, 1), F(proj_qk, 0, 1), F(proj_qk, 1, 1)],
            (1, 0): [F(cproj, 0, q) for q in range(4)],
            (1, 1): [],
            (2, 0): [F(cproj, 1, q) for q in range(4)],
            (2, 1): [],
            (3, 0): [F(cproj, 2, q) for q in range(4)],
            (3, 1): [],
        }

        def attend(j, p, pre):
            """16 i-steps for (j, pair p); pre = fillers list."""
            pvd = [
                ps.tile([P, NQ], fp32, tag="pvd", name=f"pvd{j}{p}{hh}")
                for hh in range(2)
            ]
            fi = 0
            for i in range(NI):
                sp = ps.tile([P, 2 * NQ], fp32, tag="s", name=f"s{j}{p}{i}")
                jb, ib = divmod(i, 4)
                for hh in range(2):
                    nc.tensor.matmul(
                        sp[:, hh * NQ : (hh + 1) * NQ],
                        qks[2 + p][jb][64 * hh : 64 * hh + 64, ib * P : (ib + 1) * P],
                        qks[p][j][64 * hh : 64 * hh + 64, :],
                        start=True,
                        stop=True,
                        tile_position=(64 * hh, 0),
                    )
                e = epool.tile([P, 2 * NQ], bf16, tag="e", name=f"e{j}{p}{i}")
                if i in DVE_EXP_I:
                    scr = scrp.tile([P, 2 * NQ], fp32, tag="scr", name=f"sc{j}{p}{i}")
                    nc.vector._custom_dve(
                        EXP_BASE, out=scr[:], in0=sp[:],
                        s0=1.0 / 256, s1=1.0, imm2=0.5,
                    )
                    nc.vector._custom_dve(EXP_SQ8, out=e[:], in0=scr[:])
                else:
                    nc.scalar.activation(e[:], sp[:], Exp, bias=bias0[:, 0:1])
                # interleave up to 2 fillers per step, before the PVs
                for _ in range(2):
                    if fi < len(pre):
                        pre[fi]()
                        fi += 1
                for hh in range(2):
                    nc.tensor.matmul(
                        pvd[hh][:],
                        vts[i][:, (2 * p + hh) * P : (2 * p + hh + 1) * P],
                        e[:, hh * NQ : (hh + 1) * NQ],
                        start=(i == 0),
                        stop=(i == NI - 1),
                    )
            while fi < len(pre):
                pre[fi]()
                fi += 1
            # normalize: 1/denom from psum row 64, broadcast, multiply
            for hh in range(2):
                h = 2 * p + hh
                pvs = pvsp.tile([64, NQ], fp32, tag="pvs", name=f"pvs{j}{p}{hh}")
                nc.vector.tensor_copy(pvs[:], pvd[hh][0:64, :])
                dh = rcpp.tile([1, NQ], fp32, tag="d", name=f"dh{j}{p}{hh}")
                # custom DVE ops don't shift partitions: stage row 64 to p0
                nc.vector.tensor_copy(dh[:], pvd[hh][64:65, :])
                rh = rcpp.tile([1, NQ], fp32, tag="r", name=f"rh{j}{p}{hh}")
                nc.vector.reciprocal_approx_fast(out=rh[:], in_=dh[:])
                bc = rcpp.tile([64, NQ], fp32, tag="b", name=f"bc{j}{p}{hh}")
                nc.gpsimd.partition_broadcast(bc[:], rh[0:1, :])
                nc.vector.tensor_mul(ys[p][j][64 * hh : 64 * hh + 64, :], pvs[:], bc[:])

        # ---- main schedule ----
        proj_qk(2, 0)   # k pair0, j01
        proj_qk(0, 0)   # q pair0, j01
        proj_v(0)
        proj_v(1)
        proj_qk(2, 1)   # k pair0, j23 (keys 1024..2047 for every group)
        for j in range(NJ):
            for p in range(2):
                attend(j, p, fillers[(j, p)])
        for q in range(4):
            cproj(3, q)


def _get_nc():
    if "nc" not in _CACHE:
        _CACHE["nc"] = _build()
    return _CACHE["nc"]


def _make_in_maps(x, W_attn, b_attn, W_proj):
    import ml_dtypes

    bf = ml_dtypes.bfloat16
    x = np.asarray(x, np.float32)
    W_attn = np.asarray(W_attn, np.float32)
    b_attn = np.asarray(b_attn, np.float32)
    scale = 1.0 / np.sqrt(np.float32(HD))
    W_proj = np.asarray(W_proj, np.float32)
    in_maps = []
    for c in range(NCORES):
        b, g = divmod(c, 4)
        sl = slice(FG * g, FG * (g + 1))
        wq = W_attn[:, sl] * scale
        wk = W_attn[:, H:][:, sl]
        wv = W_attn[:, 2 * H :][:, sl]
        in_maps.append(
            {
                "xT": np.ascontiguousarray(x[b].T).astype(bf),
                "wqkv": np.ascontiguousarray(
                    np.concatenate([wq, wk, wv], axis=1)
                ).astype(bf),
                "bqk": np.concatenate(
                    [b_attn[sl] * scale, b_attn[H:][sl]]
                ).astype(np.float32),
                "wp": np.ascontiguousarray(W_proj[sl, :]).astype(bf),
            }
        )
    return in_maps


def _gather(results, b_attn, W_proj, b_proj):
    b_attn = np.asarray(b_attn, np.float64)
    W_proj = np.asarray(W_proj, np.float64)
    b_proj = np.asarray(b_proj, np.float64)
    # v-bias commutes through softmax: y = sum_k p_k (v_k + bv) = y0 + bv
    host_bias = (b_attn[2 * H :] @ W_proj + b_proj).astype(np.float32)
    y = np.empty((B, T, H), np.float32)
    for b in range(B):
        acc = results[4 * b]["out"].astype(np.float32)
        for g in range(1, 4):
            acc = acc + results[4 * b + g]["out"].astype(np.float32)
        y[b] = acc + host_bias[None, :]
    return y


def run(x, W_attn, b_attn, W_proj, b_proj, trace=False):
    from concourse.bass_utils import run_bass_kernel_spmd

    nc = _get_nc()
    in_maps = _make_in_maps(x, W_attn, b_attn, W_proj)
    res = run_bass_kernel_spmd(nc, in_maps, list(range(NCORES)), trace=trace)
    return _gather(res.results, b_attn, W_proj, b_proj), res


def kernel(x, W_attn, b_attn, W_proj, b_proj):
    y, _ = run(x, W_attn, b_attn, W_proj, b_proj, trace=False)
    return y


# revision 11
# speedup vs baseline: 1.1007x; 1.0415x over previous
"""Causal self-attention (non-masked softmax) for TRN2, 8 NeuronCores.

Sharding: 2-way data parallel over batch x 4-way tensor parallel over heads.
Core c: batch c//4, head group c%4 (4 heads, 256 features). Host sums the
4 row-parallel c_proj partials per batch (fp16 partials) and adds the
host-folded biases (b_proj + b_attn_v @ W_proj; v-bias commutes through
softmax since sum_k p_k = 1).

Engine split per core:
  PE:  QKV projections, S^T (row-packed head pairs), PV (v augmented with a
       ones column so psum row 64 is the softmax denominator), c_proj.
  ACT: exp over S^T psum tiles [128,1024] (the kernel bottleneck) + the
       qk bias copies early (ACT idles during the projection lead-in).
  DVE: a slice of the exp tiles via two custom ops (compound base
       (1+s/256+s^2/(2*256^2)) then 8 chained squarings; ~6e-3 rel err at
       bf16 out), v staging copies, denominator reciprocal
       (reciprocal_approx_fast on the psum row), y normalize muls,
       c_proj psum->fp16 staging.
  GPSIMD: partition_broadcast of 1/denom rows, out-DMA issue.

PSUM: tag s = 2x[128,1024] (S tiles, double buffered), pvd = 2x[128,512]
(PV accumulators for the in-flight head pair), fill = 2x[128,512]
(projection / c_proj scratch) = 8 banks total.

Attention runs as 8 groups (j, pair), 16 key-chunk steps each; projection
and c_proj work is interleaved into the group streams as fillers to keep
the PE p-state high and ACT fed.
"""

import numpy as np

B, T, H, NH, HD = 2, 2048, 1024, 16, 64
P, FG = 128, 256
NQ = 512          # query block per j
NJ = 4            # T/NQ
NI = 16           # key chunks of 128
KH = 8            # hidden chunks of 128
W3 = 3 * FG       # wqkv row width per core (768)
NCORES = 8

# i-steps per (j,pair) group whose exp runs on DVE instead of ACT
DVE_EXP_I = (5, 10, 15)

_CACHE = {}


def _register_dve_exp():
    """Register the custom DVE exp ops (idempotent). exp(s) ~= base^(256)
    with base = 1 + s/256 + (s/256)^2/2: op1 computes the base, op2 squares
    eight times. Validated on hw: max rel err ~6e-3 over s in [-8, 8]."""
    import concourse.dve_ops as dve_ops
    from concourse.dve_ops import DveOp, OPS, CUSTOM_DVE_SPECS, _SUB_OPCODE_FOR_NAME
    from concourse.dve_spec import Spec, Src0, C0, C1, C2, lower, sq
    from concourse.dve_uop import DveOpSpec

    have = {o.name: o for o in OPS}
    if "EXP_BASE_XK" in have:
        return have["EXP_BASE_XK"], have["EXP_SQ8_XK"]

    def make(name, body, ref):
        spec = Spec(body=body, reference=ref)
        row = max(_SUB_OPCODE_FOR_NAME.values()) + 1
        _SUB_OPCODE_FOR_NAME[name] = row
        shas = {}
        for ver in ("v3",):
            tmp = DveOpSpec(name=name, opcode=row, uops=lower(spec, ver=ver),
                            rd1_en=False)
            shas[ver] = tmp.sha(ver)
        op = DveOp(name, spec, subdim=False, uops_sha=shas)
        OPS.append(op)
        CUSTOM_DVE_SPECS[name] = spec
        return op

    w = Src0 * C0
    base_body = C1 + w * (C1 + w * C2)

    def ref_base(in0, in1, s0, s1, imm2):
        ww = in0.astype(np.float32) * np.float32(s0)
        return (np.float32(s1) + ww * (np.float32(s1) + ww * np.float32(imm2))
                ).astype(np.float32)

    x = Src0
    for _ in range(8):
        x = sq(x)

    def ref_sq8(in0, in1, s0, s1, imm2):
        v = in0.astype(np.float32)
        for _ in range(8):
            v = (v * v).astype(np.float32)
        return v

    return make("EXP_BASE_XK", base_body, ref_base), make("EXP_SQ8_XK", x, ref_sq8)


def _build():
    import concourse.bacc as bacc
    import concourse.mybir as mybir
    import concourse.tile as tile

    EXP_BASE, EXP_SQ8 = _register_dve_exp()

    fp32 = mybir.dt.float32
    fp16 = mybir.dt.float16
    bf16 = mybir.dt.bfloat16

    nc = bacc.Bacc("TRN2", debug=False)
    xT = nc.dram_tensor("xT", [H, T], bf16, kind="ExternalInput").ap()
    wqkv = nc.dram_tensor("wqkv", [H, W3], bf16, kind="ExternalInput").ap()
    bqk = nc.dram_tensor("bqk", [2 * FG], fp32, kind="ExternalInput").ap()
    wp = nc.dram_tensor("wp", [FG, H], bf16, kind="ExternalInput").ap()
    out = nc.dram_tensor("out", [T, H], fp16, kind="ExternalOutput").ap()

    with tile.TileContext(nc) as tc:
        _emit(nc, tc, mybir, EXP_BASE, EXP_SQ8, xT, wqkv, bqk, wp, out)
    nc.compile()
    return nc


def _emit(nc, tc, mybir, EXP_BASE, EXP_SQ8, xT, wqkv, bqk, wp, out):
    from contextlib import ExitStack

    fp32 = mybir.dt.float32
    fp16 = mybir.dt.float16
    bf16 = mybir.dt.bfloat16
    Exp = mybir.ActivationFunctionType.Exp
    Ident = mybir.ActivationFunctionType.Identity

    with ExitStack() as ctx:
        pool = lambda name, bufs=1, space="SBUF": ctx.enter_context(
            tc.tile_pool(name=name, bufs=bufs, space=space)
        )

        # ---- persistent SBUF tiles (fine-grained so deps stay fine) ----
        const = pool("const")
        bias0 = const.tile([P, 1], fp32)
        nc.vector.memset(bias0[:], 0.0)
        junk = const.tile([P, NQ], bf16)        # PE warmup operand
        nc.vector.memset(junk[:], 0.001)
        warm = const.tile([P, 8], fp32)
        nc.vector.memset(warm[:], 0.0)
        bqk_sb = const.tile([P, 4], fp32)       # bias col per m-chunk
        nc.sync.dma_start(bqk_sb[:], bqk.rearrange("(m p) -> p m", p=P))

        xtp = pool("xt")
        # per (hidden chunk, token half) so early readers don't wait all DMA
        xts = [[xtp.tile([P, T // 2], bf16, name=f"xt{k}_{h}") for h in range(2)] for k in range(KH)]
        wqp = pool("wq")
        wqs = [wqp.tile([P, W3], bf16, name=f"wq{k}") for k in range(KH)]
        wpp = pool("wpp")
        wps = [wpp.tile([P, H], bf16, name=f"wp{k}") for k in range(2)]
        qkp = pool("qk")
        qks = [[qkp.tile([P, NQ], bf16, name=f"qk{m}_{j}") for j in range(NJ)] for m in range(4)]
        vp = pool("v")
        vts = [vp.tile([P, 4 * P], bf16, name=f"v{t}") for t in range(NI)]
        yp = pool("y")
        ys = [[yp.tile([P, NQ], bf16, name=f"y{k}_{j}") for j in range(NJ)] for k in range(2)]

        # ---- working pools ----
        epool = pool("e", bufs=4)
        scrp = pool("scr", bufs=2)      # DVE exp fp32 intermediate
        pvsp = pool("pvs", bufs=4)      # staged PV numerators
        rcpp = pool("rcp", bufs=4)      # 1/denom rows + broadcasts
        outp = pool("o", bufs=4)        # fp16 out staging
        ps = ctx.enter_context(tc.tile_pool(name="ps", bufs=2, space="PSUM"))

        # ---- warmups (off critical path): ACT exp table, PE p-state ----
        nc.scalar.activation(warm[:], warm[:], Exp, bias=bias0[:, 0:1])
        for t in range(NI):  # mark ones col + zero pad of v tiles
            v4 = vts[t].rearrange("p (h c) -> p h c", h=4, c=P)
            nc.vector.memset(v4[:, :, 64:66], 0.0)
            nc.vector.memset(v4[:, :, 64:65], 1.0)

        # ---- input DMAs, spread across idle engine queues ----
        dq = [nc.sync, nc.scalar, nc.gpsimd]
        di = [0]

        def dma(dst, src):
            dq[di[0] % 3].dma_start(dst, src)
            di[0] += 1

        for k in range(KH):  # critical prefix: w chunk + first token half
            dma(wqs[k][:], wqkv[k * P : (k + 1) * P, :])
            dma(xts[k][0][:], xT[k * P : (k + 1) * P, 0 : 2 * NQ])
        for k in range(KH):
            dma(xts[k][1][:], xT[k * P : (k + 1) * P, 2 * NQ :])
        for kk in range(2):
            dma(wps[kk][:], wp[kk * P : (kk + 1) * P, :])

        # PE warmup: ~14 junk matmuls to ramp the p-state while DMA lands
        for w in range(14):
            pw = ps.tile([P, NQ], fp32, tag="fill", name=f"warm{w}")
            nc.tensor.matmul(pw[:], junk[:, 0:P], junk[:], start=True, stop=True)

        # ---- building blocks ----
        def proj_qk(m, jpair):
            """q/k projection for feature chunk m, j-blocks (2*jpair, +1).
            Weights per (m,k) load once and serve both j matmuls; bias-add
            copies run on ACT (idle early) -> qk tiles."""
            pj = [
                ps.tile([P, NQ], fp32, tag="fill", name=f"qk{m}_{jpair}_{jj}")
                for jj in range(2)
            ]
            for k in range(KH):
                for jj in range(2):
                    nc.tensor.matmul(
                        pj[jj][:],
                        wqs[k][:, m * P : (m + 1) * P],
                        xts[k][jpair][:, jj * NQ : (jj + 1) * NQ],
                        start=(k == 0),
                        stop=(k == KH - 1),
                    )
            for jj in range(2):
                j = 2 * jpair + jj
                nc.scalar.activation(
                    qks[m][j][:], pj[jj][:], Ident, bias=bqk_sb[:, m : m + 1]
                )

        def proj_v(t):
            """v for token chunk t -> vts[t] cols [64h,64h+64) per head."""
            pv = ps.tile([P, NQ], fp32, tag="fill", name=f"v{t}")
            th, tl = divmod(t, 8)
            for k in range(KH):
                nc.tensor.matmul(
                    pv[:, 0:FG],
                    xts[k][th][:, tl * P : (tl + 1) * P],
                    wqs[k][:, 2 * FG :],
                    start=(k == 0),
                    stop=(k == KH - 1),
                )
            v4 = vts[t].rearrange("p (h c) -> p h c", h=4, c=P)
            nc.vector.tensor_copy(
                v4[:, :, 0:64], pv[:, 0:FG].rearrange("p (h c) -> p h c", h=4, c=64)
            )

        def cproj(j, mql):
            """c_proj for token rows j*512+128*mql."""
            pcs = [
                ps.tile([P, NQ], fp32, tag="fill", name=f"c{j}_{mql}_{n}")
                for n in range(2)
            ]
            for kk in range(2):
                for n in range(2):
                    nc.tensor.matmul(
                        pcs[n][:],
                        ys[kk][j][:, mql * P : (mql + 1) * P],
                        wps[kk][:, n * NQ : (n + 1) * NQ],
                        start=(kk == 0),
                        stop=(kk == 1),
                    )
            r0 = j * NQ + mql * P
            for n in range(2):
                ot = outp.tile([P, NQ], fp16, tag="o", name=f"ot{j}_{mql}_{n}")
                nc.vector.tensor_copy(ot[:], pcs[n][:])
                nc.sync.dma_start(out[r0 : r0 + P, n * NQ : (n + 1) * NQ], ot[:])

        # ---- filler schedule: callables interleaved into group streams ----
        def F(fn, *a):
            return lambda: fn(*a)

        fillers = {
            (0, 0): [F(proj_v, t) for t in range(2, 16)]
            + [F(proj_qk, 3, 0), F(proj_qk, 1, 0)],
            (0, 1): [F(proj_qk, 3, 1), F(proj_qk, 0, 1), F(proj_qk, 1, 1)],
            (1, 0): [F(cproj, 0, q) for q in range(4)],
            (1, 1): [],
            (2, 0): [F(cproj, 1, q) for q in range(4)],
            (2, 1): [],
            (3, 0): [F(cproj, 2, q) for q in range(4)],
            (3, 1): [],
        }

        def attend(j, p, pre):
            """16 i-steps for (j, pair p); pre = fillers list."""
            pvd = [
                ps.tile([P, NQ], fp32, tag="pvd", name=f"pvd{j}{p}{hh}")
                for hh in range(2)
            ]
            def emit_s(i):
                sp = ps.tile([P, 2 * NQ], fp32, tag="s", name=f"s{j}{p}{i}")
                jb, ib = divmod(i, 4)
                for hh in range(2):
                    nc.tensor.matmul(
                        sp[:, hh * NQ : (hh + 1) * NQ],
                        qks[2 + p][jb][64 * hh : 64 * hh + 64, ib * P : (ib + 1) * P],
                        qks[p][j][64 * hh : 64 * hh + 64, :],
                        start=True,
                        stop=True,
                        tile_position=(64 * hh, 0),
                    )
                return sp

            def emit_exp(i, sp):
                e = epool.tile([P, 2 * NQ], bf16, tag="e", name=f"e{j}{p}{i}")
                if i in DVE_EXP_I:
                    scr = scrp.tile([P, 2 * NQ], fp32, tag="scr", name=f"sc{j}{p}{i}")
                    nc.vector._custom_dve(
                        EXP_BASE, out=scr[:], in0=sp[:],
                        s0=1.0 / 256, s1=1.0, imm2=0.5,
                    )
                    nc.vector._custom_dve(EXP_SQ8, out=e[:], in0=scr[:])
                else:
                    nc.scalar.activation(e[:], sp[:], Exp, bias=bias0[:, 0:1])
                return e

            # software-pipelined emission: S(i+1) goes into the PE queue
            # BEFORE PV(i) so the in-order PE never blocks ACT's next exp.
            fi = 0
            sp = emit_s(0)
            for i in range(NI):
                e = emit_exp(i, sp)
                if i + 1 < NI:
                    sp = emit_s(i + 1)
                for _ in range(2):
                    if fi < len(pre):
                        pre[fi]()
                        fi += 1
                for hh in range(2):
                    nc.tensor.matmul(
                        pvd[hh][:],
                        vts[i][:, (2 * p + hh) * P : (2 * p + hh + 1) * P],
                        e[:, hh * NQ : (hh + 1) * NQ],
                        start=(i == 0),
                        stop=(i == NI - 1),
                    )
            while fi < len(pre):
                pre[fi]()
                fi += 1
            # normalize: 1/denom from psum row 64, broadcast, multiply
            for hh in range(2):
                h = 2 * p + hh
                pvs = pvsp.tile([64, NQ], fp32, tag="pvs", name=f"pvs{j}{p}{hh}")
                nc.vector.tensor_copy(pvs[:], pvd[hh][0:64, :])
                dh = rcpp.tile([1, NQ], fp32, tag="d", name=f"dh{j}{p}{hh}")
                # custom DVE ops don't shift partitions: stage row 64 to p0
                nc.vector.tensor_copy(dh[:], pvd[hh][64:65, :])
                rh = rcpp.tile([1, NQ], fp32, tag="r", name=f"rh{j}{p}{hh}")
                nc.vector.reciprocal_approx_fast(out=rh[:], in_=dh[:])
                bc = rcpp.tile([64, NQ], fp32, tag="b", name=f"bc{j}{p}{hh}")
                nc.gpsimd.partition_broadcast(bc[:], rh[0:1, :])
                nc.vector.tensor_mul(ys[p][j][64 * hh : 64 * hh + 64, :], pvs[:], bc[:])

        # ---- main schedule ----
        proj_qk(2, 0)   # k pair0, j01
        proj_qk(0, 0)   # q pair0, j01
        proj_v(0)
        proj_v(1)
        proj_qk(2, 1)   # k pair0, j23 (keys 1024..2047 for every group)
        for j in range(NJ):
            for p in range(2):
                attend(j, p, fillers[(j, p)])
        for q in range(4):
            cproj(3, q)


def _get_nc():
    if "nc" not in _CACHE:
        _CACHE["nc"] = _build()
    return _CACHE["nc"]


def _make_in_maps(x, W_attn, b_attn, W_proj):
    import ml_dtypes

    bf = ml_dtypes.bfloat16
    x = np.asarray(x, np.float32)
    W_attn = np.asarray(W_attn, np.float32)
    b_attn = np.asarray(b_attn, np.float32)
    scale = 1.0 / np.sqrt(np.float32(HD))
    W_proj = np.asarray(W_proj, np.float32)
    in_maps = []
    for c in range(NCORES):
        b, g = divmod(c, 4)
        sl = slice(FG * g, FG * (g + 1))
        wq = W_attn[:, sl] * scale
        wk = W_attn[:, H:][:, sl]
        wv = W_attn[:, 2 * H :][:, sl]
        in_maps.append(
            {
                "xT": np.ascontiguousarray(x[b].T).astype(bf),
                "wqkv": np.ascontiguousarray(
                    np.concatenate([wq, wk, wv], axis=1)
                ).astype(bf),
                "bqk": np.concatenate(
                    [b_attn[sl] * scale, b_attn[H:][sl]]
                ).astype(np.float32),
                "wp": np.ascontiguousarray(W_proj[sl, :]).astype(bf),
            }
        )
    return in_maps


def _gather(results, b_attn, W_proj, b_proj):
    b_attn = np.asarray(b_attn, np.float64)
    W_proj = np.asarray(W_proj, np.float64)
    b_proj = np.asarray(b_proj, np.float64)
    # v-bias commutes through softmax: y = sum_k p_k (v_k + bv) = y0 + bv
    host_bias = (b_attn[2 * H :] @ W_proj + b_proj).astype(np.float32)
    y = np.empty((B, T, H), np.float32)
    for b in range(B):
        acc = results[4 * b]["out"].astype(np.float32)
        for g in range(1, 4):
            acc = acc + results[4 * b + g]["out"].astype(np.float32)
        y[b] = acc + host_bias[None, :]
    return y


def run(x, W_attn, b_attn, W_proj, b_proj, trace=False):
    from concourse.bass_utils import run_bass_kernel_spmd

    nc = _get_nc()
    in_maps = _make_in_maps(x, W_attn, b_attn, W_proj)
    res = run_bass_kernel_spmd(nc, in_maps, list(range(NCORES)), trace=trace)
    return _gather(res.results, b_attn, W_proj, b_proj), res


def kernel(x, W_attn, b_attn, W_proj, b_proj):
    y, _ = run(x, W_attn, b_attn, W_proj, b_proj, trace=False)
    return y


# revision 12
# speedup vs baseline: 1.2026x; 1.0926x over previous
"""Causal self-attention (non-masked softmax) for TRN2, 8 NeuronCores.

Sharding: 2-way data parallel over batch x 4-way tensor parallel over heads.
Core c: batch c//4, head group c%4 (4 heads, 256 features). Host sums the
4 row-parallel c_proj partials per batch (fp16 partials) and adds the
host-folded biases (b_proj + b_attn_v @ W_proj; v-bias commutes through
softmax since sum_k p_k = 1).

Engine split per core:
  PE:  QKV projections, S^T (row-packed head pairs), PV (v augmented with a
       ones column so psum row 64 is the softmax denominator), c_proj.
  ACT: exp over S^T psum tiles [128,1024] (the kernel bottleneck) + the
       qk bias copies early (ACT idles during the projection lead-in).
  DVE: a slice of the exp tiles via two custom ops (compound base
       (1+s/256+s^2/(2*256^2)) then 8 chained squarings; ~6e-3 rel err at
       bf16 out), v staging copies, denominator reciprocal
       (reciprocal_approx_fast on the psum row), y normalize muls,
       c_proj psum->fp16 staging.
  GPSIMD: partition_broadcast of 1/denom rows, out-DMA issue.

PSUM: tag s = 2x[128,1024] (S tiles, double buffered), pvd = 2x[128,512]
(PV accumulators for the in-flight head pair), fill = 2x[128,512]
(projection / c_proj scratch) = 8 banks total.

Attention runs as 8 groups (j, pair), 16 key-chunk steps each; projection
and c_proj work is interleaved into the group streams as fillers to keep
the PE p-state high and ACT fed.
"""

import numpy as np

B, T, H, NH, HD = 2, 2048, 1024, 16, 64
P, FG = 128, 256
NQ = 512          # query block per j
NJ = 4            # T/NQ
NI = 16           # key chunks of 128
KH = 8            # hidden chunks of 128
W3 = 3 * FG       # wqkv row width per core (768)
NCORES = 8

# i-steps per (j,pair) group whose exp runs on DVE instead of ACT
DVE_EXP_I = ()

_CACHE = {}


def _register_dve_exp():
    """Register the custom DVE exp ops (idempotent). exp(s) ~= base^(256)
    with base = 1 + s/256 + (s/256)^2/2: op1 computes the base, op2 squares
    eight times. Validated on hw: max rel err ~6e-3 over s in [-8, 8]."""
    import concourse.dve_ops as dve_ops
    from concourse.dve_ops import DveOp, OPS, CUSTOM_DVE_SPECS, _SUB_OPCODE_FOR_NAME
    from concourse.dve_spec import Spec, Src0, C0, C1, C2, lower, sq
    from concourse.dve_uop import DveOpSpec

    have = {o.name: o for o in OPS}
    if "EXP_BASE_XK" in have:
        return have["EXP_BASE_XK"], have["EXP_SQ8_XK"]

    def make(name, body, ref):
        spec = Spec(body=body, reference=ref)
        row = max(_SUB_OPCODE_FOR_NAME.values()) + 1
        _SUB_OPCODE_FOR_NAME[name] = row
        shas = {}
        for ver in ("v3",):
            tmp = DveOpSpec(name=name, opcode=row, uops=lower(spec, ver=ver),
                            rd1_en=False)
            shas[ver] = tmp.sha(ver)
        op = DveOp(name, spec, subdim=False, uops_sha=shas)
        OPS.append(op)
        CUSTOM_DVE_SPECS[name] = spec
        return op

    w = Src0 * C0
    base_body = C1 + w * (C1 + w * C2)

    def ref_base(in0, in1, s0, s1, imm2):
        ww = in0.astype(np.float32) * np.float32(s0)
        return (np.float32(s1) + ww * (np.float32(s1) + ww * np.float32(imm2))
                ).astype(np.float32)

    x = Src0
    for _ in range(8):
        x = sq(x)

    def ref_sq8(in0, in1, s0, s1, imm2):
        v = in0.astype(np.float32)
        for _ in range(8):
            v = (v * v).astype(np.float32)
        return v

    return make("EXP_BASE_XK", base_body, ref_base), make("EXP_SQ8_XK", x, ref_sq8)


def _build():
    import concourse.bacc as bacc
    import concourse.mybir as mybir
    import concourse.tile as tile

    EXP_BASE, EXP_SQ8 = _register_dve_exp()

    fp32 = mybir.dt.float32
    fp16 = mybir.dt.float16
    bf16 = mybir.dt.bfloat16

    nc = bacc.Bacc("TRN2", debug=False)
    xT = nc.dram_tensor("xT", [H, T], bf16, kind="ExternalInput").ap()
    wqkv = nc.dram_tensor("wqkv", [H, W3], bf16, kind="ExternalInput").ap()
    bqk = nc.dram_tensor("bqk", [2 * FG], fp32, kind="ExternalInput").ap()
    wp = nc.dram_tensor("wp", [FG, H], bf16, kind="ExternalInput").ap()
    out = nc.dram_tensor("out", [T, H], fp16, kind="ExternalOutput").ap()

    with tile.TileContext(nc) as tc:
        _emit(nc, tc, mybir, EXP_BASE, EXP_SQ8, xT, wqkv, bqk, wp, out)
    nc.compile()
    return nc


def _emit(nc, tc, mybir, EXP_BASE, EXP_SQ8, xT, wqkv, bqk, wp, out):
    from contextlib import ExitStack

    fp32 = mybir.dt.float32
    fp16 = mybir.dt.float16
    bf16 = mybir.dt.bfloat16
    Exp = mybir.ActivationFunctionType.Exp
    Ident = mybir.ActivationFunctionType.Identity

    with ExitStack() as ctx:
        pool = lambda name, bufs=1, space="SBUF": ctx.enter_context(
            tc.tile_pool(name=name, bufs=bufs, space=space)
        )

        # ---- persistent SBUF tiles (fine-grained so deps stay fine) ----
        const = pool("const")
        bias0 = const.tile([P, 1], fp32)
        nc.vector.memset(bias0[:], 0.0)
        junk = const.tile([P, NQ], bf16)        # PE warmup operand
        nc.vector.memset(junk[:], 0.001)
        warm = const.tile([P, 8], fp32)
        nc.vector.memset(warm[:], 0.0)
        bqk_sb = const.tile([P, 4], fp32)       # bias col per m-chunk
        nc.sync.dma_start(bqk_sb[:], bqk.rearrange("(m p) -> p m", p=P))

        xtp = pool("xt")
        # per (hidden chunk, token half) so early readers don't wait all DMA
        xts = [[xtp.tile([P, T // 2], bf16, name=f"xt{k}_{h}") for h in range(2)] for k in range(KH)]
        wqp = pool("wq")
        wqs = [wqp.tile([P, W3], bf16, name=f"wq{k}") for k in range(KH)]
        wpp = pool("wpp")
        wps = [wpp.tile([P, H], bf16, name=f"wp{k}") for k in range(2)]
        qkp = pool("qk")
        qks = [[qkp.tile([P, NQ], bf16, name=f"qk{m}_{j}") for j in range(NJ)] for m in range(4)]
        vp = pool("v")
        vts = [vp.tile([P, 4 * P], bf16, name=f"v{t}") for t in range(NI)]
        yp = pool("y")
        ys = [[yp.tile([P, NQ], bf16, name=f"y{k}_{j}") for j in range(NJ)] for k in range(2)]

        # ---- working pools ----
        epool = pool("e", bufs=4)
        scrp = pool("scr", bufs=2)      # DVE exp fp32 intermediate
        pvsp = pool("pvs", bufs=4)      # staged PV numerators
        rcpp = pool("rcp", bufs=4)      # 1/denom rows + broadcasts
        outp = pool("o", bufs=4)        # fp16 out staging
        ps = ctx.enter_context(tc.tile_pool(name="ps", bufs=2, space="PSUM"))

        # ---- warmups (off critical path): ACT exp table, PE p-state ----
        nc.scalar.activation(warm[:], warm[:], Exp, bias=bias0[:, 0:1])
        for t in range(NI):  # mark ones col + zero pad of v tiles
            v4 = vts[t].rearrange("p (h c) -> p h c", h=4, c=P)
            nc.vector.memset(v4[:, :, 64:66], 0.0)
            nc.vector.memset(v4[:, :, 64:65], 1.0)

        # ---- input DMAs, spread across idle engine queues ----
        dq = [nc.sync, nc.scalar, nc.gpsimd]
        di = [0]

        def dma(dst, src):
            dq[di[0] % 3].dma_start(dst, src)
            di[0] += 1

        for k in range(KH):  # critical prefix: w chunk + first token half
            dma(wqs[k][:], wqkv[k * P : (k + 1) * P, :])
            dma(xts[k][0][:], xT[k * P : (k + 1) * P, 0 : 2 * NQ])
        for k in range(KH):
            dma(xts[k][1][:], xT[k * P : (k + 1) * P, 2 * NQ :])
        for kk in range(2):
            dma(wps[kk][:], wp[kk * P : (kk + 1) * P, :])

        # PE warmup: ~14 junk matmuls to ramp the p-state while DMA lands
        for w in range(14):
            pw = ps.tile([P, NQ], fp32, tag="fill", name=f"warm{w}")
            nc.tensor.matmul(pw[:], junk[:, 0:P], junk[:], start=True, stop=True)

        # ---- building blocks ----
        def proj_qk(m, jpair):
            """q/k projection for feature chunk m, j-blocks (2*jpair, +1).
            Weights per (m,k) load once and serve both j matmuls; bias-add
            copies run on ACT (idle early) -> qk tiles."""
            pj = [
                ps.tile([P, NQ], fp32, tag="fill", name=f"qk{m}_{jpair}_{jj}")
                for jj in range(2)
            ]
            for k in range(KH):
                for jj in range(2):
                    nc.tensor.matmul(
                        pj[jj][:],
                        wqs[k][:, m * P : (m + 1) * P],
                        xts[k][jpair][:, jj * NQ : (jj + 1) * NQ],
                        start=(k == 0),
                        stop=(k == KH - 1),
                    )
            for jj in range(2):
                j = 2 * jpair + jj
                nc.scalar.activation(
                    qks[m][j][:], pj[jj][:], Ident, bias=bqk_sb[:, m : m + 1]
                )

        def proj_v(t):
            """v for token chunk t -> vts[t] cols [64h,64h+64) per head."""
            pv = ps.tile([P, NQ], fp32, tag="fill", name=f"v{t}")
            th, tl = divmod(t, 8)
            for k in range(KH):
                nc.tensor.matmul(
                    pv[:, 0:FG],
                    xts[k][th][:, tl * P : (tl + 1) * P],
                    wqs[k][:, 2 * FG :],
                    start=(k == 0),
                    stop=(k == KH - 1),
                )
            v4 = vts[t].rearrange("p (h c) -> p h c", h=4, c=P)
            nc.vector.tensor_copy(
                v4[:, :, 0:64], pv[:, 0:FG].rearrange("p (h c) -> p h c", h=4, c=64)
            )

        def cproj(j, mql):
            """c_proj for token rows j*512+128*mql."""
            pcs = [
                ps.tile([P, NQ], fp32, tag="fill", name=f"c{j}_{mql}_{n}")
                for n in range(2)
            ]
            for kk in range(2):
                for n in range(2):
                    nc.tensor.matmul(
                        pcs[n][:],
                        ys[kk][j][:, mql * P : (mql + 1) * P],
                        wps[kk][:, n * NQ : (n + 1) * NQ],
                        start=(kk == 0),
                        stop=(kk == 1),
                    )
            r0 = j * NQ + mql * P
            for n in range(2):
                ot = outp.tile([P, NQ], fp16, tag="o", name=f"ot{j}_{mql}_{n}")
                nc.vector.tensor_copy(ot[:], pcs[n][:])
                nc.sync.dma_start(out[r0 : r0 + P, n * NQ : (n + 1) * NQ], ot[:])

        # ---- filler schedule: callables interleaved into group streams ----
        def F(fn, *a):
            return lambda: fn(*a)

        fillers = {
            (0, 0): [F(proj_v, t) for t in range(2, 16)]
            + [F(proj_qk, 3, 0), F(proj_qk, 1, 0)],
            (0, 1): [F(proj_qk, 3, 1), F(proj_qk, 0, 1), F(proj_qk, 1, 1)],
            (1, 0): [F(cproj, 0, q) for q in range(4)],
            (1, 1): [],
            (2, 0): [F(cproj, 1, q) for q in range(4)],
            (2, 1): [],
            (3, 0): [F(cproj, 2, q) for q in range(4)],
            (3, 1): [],
        }

        def attend(j, p, pre):
            """16 i-steps for (j, pair p); pre = fillers list."""
            pvd = [
                ps.tile([P, NQ], fp32, tag="pvd", name=f"pvd{j}{p}{hh}")
                for hh in range(2)
            ]
            def emit_s(i):
                sp = ps.tile([P, 2 * NQ], fp32, tag="s", name=f"s{j}{p}{i}")
                jb, ib = divmod(i, 4)
                for hh in range(2):
                    nc.tensor.matmul(
                        sp[:, hh * NQ : (hh + 1) * NQ],
                        qks[2 + p][jb][64 * hh : 64 * hh + 64, ib * P : (ib + 1) * P],
                        qks[p][j][64 * hh : 64 * hh + 64, :],
                        start=True,
                        stop=True,
                        tile_position=(64 * hh, 0),
                    )
                return sp

            def emit_exp(i, sp):
                e = epool.tile([P, 2 * NQ], bf16, tag="e", name=f"e{j}{p}{i}")
                if i in DVE_EXP_I:
                    scr = scrp.tile([P, 2 * NQ], fp32, tag="scr", name=f"sc{j}{p}{i}")
                    nc.vector._custom_dve(
                        EXP_BASE, out=scr[:], in0=sp[:],
                        s0=1.0 / 256, s1=1.0, imm2=0.5,
                    )
                    nc.vector._custom_dve(EXP_SQ8, out=e[:], in0=scr[:])
                else:
                    nc.scalar.activation(e[:], sp[:], Exp, bias=bias0[:, 0:1])
                return e

            # software-pipelined emission: S(i+1) goes into the PE queue
            # BEFORE PV(i) so the in-order PE never blocks ACT's next exp.
            fi = 0
            sp = emit_s(0)
            for i in range(NI):
                e = emit_exp(i, sp)
                if i + 1 < NI:
                    sp = emit_s(i + 1)
                for _ in range(2):
                    if fi < len(pre):
                        pre[fi]()
                        fi += 1
                for hh in range(2):
                    nc.tensor.matmul(
                        pvd[hh][:],
                        vts[i][:, (2 * p + hh) * P : (2 * p + hh + 1) * P],
                        e[:, hh * NQ : (hh + 1) * NQ],
                        start=(i == 0),
                        stop=(i == NI - 1),
                    )
            while fi < len(pre):
                pre[fi]()
                fi += 1
            # normalize: 1/denom from psum row 64, broadcast, multiply
            for hh in range(2):
                h = 2 * p + hh
                pvs = pvsp.tile([64, NQ], fp32, tag="pvs", name=f"pvs{j}{p}{hh}")
                nc.vector.tensor_copy(pvs[:], pvd[hh][0:64, :])
                dh = rcpp.tile([1, NQ], fp32, tag="d", name=f"dh{j}{p}{hh}")
                # custom DVE ops don't shift partitions: stage row 64 to p0
                nc.vector.tensor_copy(dh[:], pvd[hh][64:65, :])
                rh = rcpp.tile([1, NQ], fp32, tag="r", name=f"rh{j}{p}{hh}")
                nc.vector.reciprocal_approx_fast(out=rh[:], in_=dh[:])
                bc = rcpp.tile([64, NQ], fp32, tag="b", name=f"bc{j}{p}{hh}")
                nc.gpsimd.partition_broadcast(bc[:], rh[0:1, :])
                nc.vector.tensor_mul(ys[p][j][64 * hh : 64 * hh + 64, :], pvs[:], bc[:])

        # ---- main schedule ----
        proj_qk(2, 0)   # k pair0, j01
        proj_qk(0, 0)   # q pair0, j01
        proj_v(0)
        proj_v(1)
        proj_qk(2, 1)   # k pair0, j23 (keys 1024..2047 for every group)
        for j in range(NJ):
            for p in range(2):
                attend(j, p, fillers[(j, p)])
        for q in range(4):
            cproj(3, q)


def _get_nc():
    if "nc" not in _CACHE:
        _CACHE["nc"] = _build()
    return _CACHE["nc"]


def _make_in_maps(x, W_attn, b_attn, W_proj):
    import ml_dtypes

    bf = ml_dtypes.bfloat16
    x = np.asarray(x, np.float32)
    W_attn = np.asarray(W_attn, np.float32)
    b_attn = np.asarray(b_attn, np.float32)
    scale = 1.0 / np.sqrt(np.float32(HD))
    W_proj = np.asarray(W_proj, np.float32)
    in_maps = []
    for c in range(NCORES):
        b, g = divmod(c, 4)
        sl = slice(FG * g, FG * (g + 1))
        wq = W_attn[:, sl] * scale
        wk = W_attn[:, H:][:, sl]
        wv = W_attn[:, 2 * H :][:, sl]
        in_maps.append(
            {
                "xT": np.ascontiguousarray(x[b].T).astype(bf),
                "wqkv": np.ascontiguousarray(
                    np.concatenate([wq, wk, wv], axis=1)
                ).astype(bf),
                "bqk": np.concatenate(
                    [b_attn[sl] * scale, b_attn[H:][sl]]
                ).astype(np.float32),
                "wp": np.ascontiguousarray(W_proj[sl, :]).astype(bf),
            }
        )
    return in_maps


def _gather(results, b_attn, W_proj, b_proj):
    b_attn = np.asarray(b_attn, np.float64)
    W_proj = np.asarray(W_proj, np.float64)
    b_proj = np.asarray(b_proj, np.float64)
    # v-bias commutes through softmax: y = sum_k p_k (v_k + bv) = y0 + bv
    host_bias = (b_attn[2 * H :] @ W_proj + b_proj).astype(np.float32)
    y = np.empty((B, T, H), np.float32)
    for b in range(B):
        acc = results[4 * b]["out"].astype(np.float32)
        for g in range(1, 4):
            acc = acc + results[4 * b + g]["out"].astype(np.float32)
        y[b] = acc + host_bias[None, :]
    return y


def run(x, W_attn, b_attn, W_proj, b_proj, trace=False):
    from concourse.bass_utils import run_bass_kernel_spmd

    nc = _get_nc()
    in_maps = _make_in_maps(x, W_attn, b_attn, W_proj)
    res = run_bass_kernel_spmd(nc, in_maps, list(range(NCORES)), trace=trace)
    return _gather(res.results, b_attn, W_proj, b_proj), res


def kernel(x, W_attn, b_attn, W_proj, b_proj):
    y, _ = run(x, W_attn, b_attn, W_proj, b_proj, trace=False)
    return y
